# revision 24
# baseline (speedup 1.0000x reference)
"""ChaosAttention on 8 Trainium2 NeuronCores.

Sharding: tensor-parallel over heads. Each of the 8 cores owns H/8 = 2 heads
(128 of the 1024 q/k/v projection columns, 128 of the Wo rows). Every core
reads the full x (as x^T, bf16); the out-projection is row-parallel, so each
core returns a partial y^T (bf16) and the host sums the 8 partials and adds
bo plus the folded V-bias term (bv @ Wo).

The physics adapter (physics_state @ Wp1 -> gelu -> @ Wp2) produces a bias
that is constant along the softmax axis, so softmax(x + c) == softmax(x) and
the adapter has zero effect on the output; it is skipped entirely.

Key compaction: masked keys get -inf scores in the reference, so the host
packs only the kept keys per batch (padded to a multiple of 128; pad slots
get a -1e30 exp bias). With a ~50% random mask this halves QK^T / exp / AV.

Device kernel layout (per core):
  - scores are computed transposed (S^T: keys on partitions, queries free) so
    the pad-mask is a per-partition bias fused into the ACT-engine exp, and
    the AV matmul needs no transpose of the probs.
  - the two heads are row-packed into the PE array for QK^T via tile_position.
  - V is produced with wv stationary (big N<=512 matmuls instead of tiny
    N=128 ones) as v^T, then flipped into the [keys, dims] layout AV needs
    with cheap PE transposes (identity matmul). An appended ones-column makes
    the AV matmul also emit the softmax denominator Z in the same PSUM
    accumulation.
  - 1/Z via one merged reciprocal_approx_fast per chunk (~5x faster than DVE
    reciprocal, ~18 good bits; needs a partition-0 source, so both heads' Z
    rows are first staged into a [1, 2*IC] tile). gpsimd broadcasts it across
    the head dims; the V-bias is folded into the host-side output fixup
    (bv @ Wo), so no per-chunk bias add is needed.
  - softmax max-subtraction is skipped (scores are O(1), fp32 exp is safe).
  - y^T partials are staged to bf16 (halves the HBM write traffic); each
    write is split into two partition-halves so the final chunk's writes
    spread over more DMA queues.
  - The PE clock drops to 1.2 GHz after any idle gap and only returns to
    2.4 GHz after 3us of continuous execution, so the whole program is
    emission-ordered to keep the PE's in-order queue non-empty: batch 0 gets
    a tiny 128-key first window so attention starts within ~5us, and all
    remaining projection work is drip-fed into the attention inner loop in
    DMA-arrival order (a per-chunk pre-pop guarantees a chunk's Q projection
    is always emitted before its first QK matmul).
"""

import numpy as np
import ml_dtypes

_BF16 = ml_dtypes.bfloat16

B, T, E, H, D = 2, 2048, 1024, 16, 64
BT = B * T                 # 4096 tokens
N_CORES = 8
PCN = E // N_CORES         # 128 per-core projection dims (2 heads x 64)
EC = E // 128              # 8 contraction chunks for the projections
IC = 512                   # query-chunk size
NQC = T // IC              # 4 query chunks per batch
SCALE = 1.0 / float(np.sqrt(D))

_cache = {}


def _kwins(nkt, b):
    """Key windows for batch b as (start, size) over its packed columns.
    Batch 0 gets a tiny 128-wide first window so attention can start early."""
    n = nkt[b] * 128
    wins = []
    w0 = 0
    if b == 0:
        wins.append((0, 128))
        w0 = 128
    while w0 < n:
        sz = min(512, n - w0)
        wins.append((w0, sz))
        w0 += sz
    return wins


def _build(nkt):
    """Build + schedule the per-core Bass program. nkt = packed key-tile
    counts per batch (ceil(kept/128))."""
    key = tuple(nkt)
    if key in _cache:
        return _cache[key]

    from contextlib import ExitStack
    import concourse.tile as tile
    from concourse import bacc, mybir

    f32 = mybir.dt.float32
    bf16 = mybir.dt.bfloat16
    Exp = mybir.ActivationFunctionType.Exp

    base = [0, nkt[0]]              # packed key-tile offset per batch
    ntt = nkt[0] + nkt[1]           # total packed key tiles
    KP = ntt * 128                  # total packed+padded kv tokens

    kwins = {b: _kwins(nkt, b) for b in range(B)}
    # jt -> (window index, 128-key subtile within window)
    tmap = {b: [(wi, q) for wi, (w0, wsz) in enumerate(kwins[b])
                for q in range(wsz // 128)] for b in range(B)}

    nc = bacc.Bacc("TRN2", target_bir_lowering=False, debug=False,
                   num_devices=N_CORES)

    xT_d = nc.dram_tensor("xT", [E, BT], bf16, kind="ExternalInput").ap()
    xk_d = nc.dram_tensor("xk", [E, KP], bf16, kind="ExternalInput").ap()
    wq_d = nc.dram_tensor("wq", [128, EC * PCN], bf16, kind="ExternalInput").ap()
    wk_d = nc.dram_tensor("wk", [128, EC * PCN], bf16, kind="ExternalInput").ap()
    wv_d = nc.dram_tensor("wv", [128, EC * PCN], bf16, kind="ExternalInput").ap()
    wo_d = nc.dram_tensor("wo", [PCN, E], bf16, kind="ExternalInput").ap()
    bq_d = nc.dram_tensor("bq", [PCN, 1], f32, kind="ExternalInput").ap()
    bk_d = nc.dram_tensor("bk", [PCN, 1], f32, kind="ExternalInput").ap()
    mb_d = nc.dram_tensor("mb", [128, ntt], f32, kind="ExternalInput").ap()
    id_d = nc.dram_tensor("ident", [128, 128], bf16, kind="ExternalInput").ap()
    yT_d = nc.dram_tensor("yT", [E, BT], bf16, kind="ExternalOutput").ap()

    with tile.TileContext(nc) as tc, ExitStack() as ctx:
        consts = ctx.enter_context(tc.tile_pool(name="consts", bufs=1))
        pp_mm = ctx.enter_context(tc.tile_pool(name="ppmm", bufs=2, space="PSUM"))
        pp_st = ctx.enter_context(tc.tile_pool(name="ppst", bufs=2, space="PSUM"))
        pp_o = ctx.enter_context(tc.tile_pool(name="ppo", bufs=2, space="PSUM"))
        pool_pt = ctx.enter_context(tc.tile_pool(name="ptp", bufs=4))
        pool_vt = ctx.enter_context(tc.tile_pool(name="vtp", bufs=2))
        pool_oc = ctx.enter_context(tc.tile_pool(name="ocp", bufs=2))
        pool_rz = ctx.enter_context(tc.tile_pool(name="rzp", bufs=2))
        pool_rb = ctx.enter_context(tc.tile_pool(name="rbp", bufs=2))
        pool_y = ctx.enter_context(tc.tile_pool(name="yp", bufs=4))

        # ---- persistent SBUF residents ----
        xq = [consts.tile([128, EC, IC], bf16, tag=f"xq{mw}", name=f"xq{mw}")
              for mw in range(BT // IC)]
        xkw = {}
        for b in range(B):
            for wi, (w0, wsz) in enumerate(kwins[b]):
                xkw[(b, wi)] = consts.tile([128, EC, wsz], bf16,
                                           tag=f"xk{b}_{wi}", name=f"xk{b}_{wi}")

        wq_sb = consts.tile([128, EC, PCN], bf16, tag="wq")
        wk_sb = consts.tile([128, EC, PCN], bf16, tag="wk")
        wv_sb = consts.tile([128, EC, PCN], bf16, tag="wv")
        wo_sb = consts.tile([128, E], bf16, tag="wo")
        mb_sb = consts.tile([128, ntt], f32, tag="mb")
        bq_sb = consts.tile([128, 1], f32, tag="bq")
        bk_sb = consts.tile([128, 1], f32, tag="bk")
        id_sb = consts.tile([128, 128], bf16, tag="ident")

        # DMA piece sizes balance two costs: each dma_start occupies its
        # issuing engine ~700ns (so startup-critical issues are spread across
        # all four idle engines), and one dma_start = one queue at ~20GB/s
        # (so startup-critical data is split across many queues).
        xkT_v = xk_d.rearrange("(c p) n -> p c n", p=128)
        xTT_v = xT_d.rearrange("(c p) n -> p c n", p=128)
        _rr = {"i": 0}
        _engs = None

        def issue(dst, src):
            if _engs is None:
                nc.sync.dma_start(dst, src)
            else:
                eng = _engs[_rr["i"] % len(_engs)]
                _rr["i"] += 1
                eng.dma_start(dst, src)

        def dma_xk(b, wi, pieces):
            w0, wsz = kwins[b][wi]
            c0 = base[b] * 128 + w0
            step = EC // pieces
            for p in range(pieces):
                issue(xkw[(b, wi)][:, p * step:(p + 1) * step, :],
                      xkT_v[:, p * step:(p + 1) * step, c0:c0 + wsz])

        def dma_xq(mw, pieces):
            step = EC // pieces
            for p in range(pieces):
                issue(xq[mw][:, p * step:(p + 1) * step, :],
                      xTT_v[:, p * step:(p + 1) * step, mw * IC:(mw + 1) * IC])

        def dma_w(w_sb, w_d, pieces=2):
            w_v = w_d.rearrange("p (c n) -> p c n", n=PCN)
            step = EC // pieces
            for p in range(pieces):
                issue(w_sb[:, p * step:(p + 1) * step, :],
                      w_v[:, p * step:(p + 1) * step, :])

        # DMA emission order matches the order the PE will need the data.
        # Startup-critical stream: issue round-robin over all engines.
        _engs = [nc.sync, nc.scalar, nc.gpsimd]
        dma_w(wk_sb, wk_d)
        dma_xk(0, 0, 4)
        issue(id_sb[:], id_d[:])        # v00's PE transposes block on this
        issue(bk_sb[:], bk_d[:])
        dma_w(wq_sb, wq_d)
        dma_w(wv_sb, wv_d)
        issue(mb_sb[:], mb_d[:])
        dma_xq(0, 8)
        issue(bq_sb[:], bq_d[:])
        _engs = [nc.sync, nc.gpsimd]    # scalar starts exps soon after
        dma_xk(0, 1, 8)
        for p in range(2):
            issue(wo_sb[:, p * 512:(p + 1) * 512],
                  wo_d[:, p * 512:(p + 1) * 512])
        dma_xq(1, 8)
        _engs = None                    # prefetch stream: sync only
        for wi in range(2, len(kwins[0])):
            dma_xk(0, wi, 2)
        for wi in range(len(kwins[1])):
            dma_xk(1, wi, 4)
        dma_xq(2, 4)
        dma_xq(3, 4)
        for mw in range(4, BT // IC):
            dma_xq(mw, 2)

        # per-chunk tiles: fine-grained deps let attention start early
        qTc = {(b, icx): consts.tile([128, IC], bf16, tag=f"qT{b}_{icx}",
                                     name=f"qT{b}_{icx}")
               for b in range(B) for icx in range(NQC)}
        ATc = {(b, icx): consts.tile([128, IC], bf16, tag=f"AT{b}_{icx}",
                                     name=f"AT{b}_{icx}")
               for b in range(B) for icx in range(NQC)}
        kTc = {}
        Vpg = {}
        for b in range(B):
            for wi, (w0, wsz) in enumerate(kwins[b]):
                kTc[(b, wi)] = consts.tile([128, wsz], bf16, tag=f"kT{b}_{wi}",
                                           name=f"kT{b}_{wi}")
                Vpg[(b, wi)] = consts.tile([128, wsz // 128, 2, D + 1], bf16,
                                           tag=f"Vp{b}_{wi}", name=f"Vp{b}_{wi}")
                nc.gpsimd.memset(Vpg[(b, wi)][:, :, :, D:D + 1], 1.0)

        # ---- phase emitters ----
        q_emitted = set()

        def proj_q_chunk(b, icx):
            m0 = b * T + icx * IC
            ps = pp_mm.tile([128, 512], f32, tag="mm", name="psq")
            for ec in range(EC):
                nc.tensor.matmul(ps[:], lhsT=wq_sb[:, ec, :],
                                 rhs=xq[m0 // IC][:, ec, :],
                                 start=(ec == 0), stop=(ec == EC - 1))
            nc.vector.tensor_scalar_add(out=qTc[(b, icx)][:], in0=ps[:],
                                        scalar1=bq_sb[:])
            q_emitted.add((b, icx))

        def proj_k_chunk(b, wi, wsz):
            ps = pp_mm.tile([128, 512], f32, tag="mm", name="psk")
            for ec in range(EC):
                nc.tensor.matmul(ps[:, 0:wsz], lhsT=wk_sb[:, ec, :],
                                 rhs=xkw[(b, wi)][:, ec, :],
                                 start=(ec == 0), stop=(ec == EC - 1))
            nc.vector.tensor_scalar_add(out=kTc[(b, wi)][:], in0=ps[:, 0:wsz],
                                        scalar1=bk_sb[:])

        def proj_v_group(b, wi, wsz):
            # v^T = wv^T x (wv stationary, N=wsz), then PE-transpose back to
            # the [keys, dims] layout AV needs.
            nt = wsz // 128
            ps = pp_mm.tile([128, 512], f32, tag="mm", name="psv")
            for ec in range(EC):
                nc.tensor.matmul(ps[:, 0:wsz], lhsT=wv_sb[:, ec, :],
                                 rhs=xkw[(b, wi)][:, ec, :],
                                 start=(ec == 0), stop=(ec == EC - 1))
            vt = pool_vt.tile([128, 512], bf16, tag="vt", name="vt")
            nc.vector.tensor_copy(out=vt[:, 0:wsz], in_=ps[:, 0:wsz])
            tp = pp_mm.tile([128, 512], bf16, tag="mm", name="tpv")
            for q in range(nt):
                nc.tensor.transpose(tp[:, q * 128:(q + 1) * 128],
                                    vt[:, q * 128:(q + 1) * 128], id_sb[:])
            nc.vector.tensor_copy(
                out=Vpg[(b, wi)][:, 0:nt, :, 0:D],
                in_=tp[:, 0:wsz].rearrange("p (q h d) -> p q h d", q=nt, h=2))

        def kv_items(b, skip=0):
            items = []
            for wi, (w0, wsz) in enumerate(kwins[b]):
                items.append(lambda b=b, wi=wi, wsz=wsz: proj_k_chunk(b, wi, wsz))
                items.append(lambda b=b, wi=wi, wsz=wsz: proj_v_group(b, wi, wsz))
            return items[skip:]

        def q_item(b, icx):
            return lambda: proj_q_chunk(b, icx)

        def outproj_unit(b, icx, et, eng):
            yp = pp_mm.tile([128, 512], f32, tag="mm", name="psy")
            nc.tensor.matmul(yp[:], lhsT=wo_sb[:, et * 128:(et + 1) * 128],
                             rhs=ATc[(b, icx)][:], start=True, stop=True)
            ysb = pool_y.tile([128, 512], bf16, tag="y")
            if eng == "act":
                # last chunk only: exps are done, so ACT can help drain
                nc.scalar.activation(out=ysb[:], in_=yp[:],
                                     func=mybir.ActivationFunctionType.Copy)
            else:
                nc.vector.tensor_copy(out=ysb[:], in_=yp[:])
            i0 = b * T + icx * IC
            # issue batch-0 writes from gpsimd (sync is busy with the input
            # stream early on) and batch-1 writes from sync. The last chunk
            # is the kernel tail: alternate sync/gpsimd so neither engine's
            # ~700ns/issue serializes the final drain.
            if b == B - 1 and icx == NQC - 1:
                q = nc.sync if et % 2 else nc.gpsimd
            else:
                q = nc.sync if b else nc.gpsimd
            q.dma_start(yT_d[et * 128:(et + 1) * 128, i0:i0 + 512], ysb[:])

        def outproj_units(b, icx):
            # the last two chunks' units run after the final exp, so ACT can
            # take half their staging casts and drain in parallel with DVE;
            # the first two chunks' units run while attention is PE-paced
            # (ACT has slack there), so ACT takes part of those too.
            tail = b == B - 1 and icx >= NQC - 2
            early = b == 0 and icx < 2
            return [lambda et=et: outproj_unit(
                        b, icx, et,
                        "act" if (tail and et % 2 == icx % 2)
                        or (early and et % 2) else "dve")
                    for et in range(EC)]

        def attn_chunk(b, icx, light, light2, heavy, norm_prev, rate):
            # a chunk's Q projection must be on the PE queue before its QK
            while (b, icx) not in q_emitted and heavy:
                heavy.pop(0)()
            o_ps = [pp_o.tile([D + 1, IC], f32, tag="o", name=f"o{h}")
                    for h in range(2)]
            njt = nkt[b]
            for jt in range(njt):
                tg = base[b] + jt
                wi, q = tmap[b][jt]
                st = pp_st.tile([128, 2 * IC], f32, tag="st")
                nc.tensor.matmul(st[:, 0:IC],
                                 lhsT=kTc[(b, wi)][0:64, q * 128:(q + 1) * 128],
                                 rhs=qTc[(b, icx)][0:64, :],
                                 tile_position=(0, 0), start=True, stop=True)
                nc.tensor.matmul(st[:, IC:2 * IC],
                                 lhsT=kTc[(b, wi)][64:128, q * 128:(q + 1) * 128],
                                 rhs=qTc[(b, icx)][64:128, :],
                                 tile_position=(64, 0), start=True, stop=True)
                pt = pool_pt.tile([128, 2 * IC], bf16, tag="pt")
                nc.scalar.activation(out=pt[:], in_=st[:], func=Exp,
                                     bias=mb_sb[:, tg:tg + 1], scale=SCALE)
                for h in range(2):
                    nc.tensor.matmul(o_ps[h][:], lhsT=Vpg[(b, wi)][:, q, h, :],
                                     rhs=pt[:, h * IC:(h + 1) * IC],
                                     start=(jt == 0), stop=(jt == njt - 1))
                if jt == njt - 1:
                    break               # last jt's pops happen after the ocs
                if light:
                    light.pop(0)()
                if norm_prev and jt >= 2:
                    norm_prev.pop(0)()
                if norm_prev and jt >= 3:
                    norm_prev.pop(0)()
                # once the previous chunk is normalized, start its own
                # out-projection units (shortens the final-chunk tail)
                if light2 and jt >= 6 and not norm_prev:
                    light2.pop(0)()
                if heavy and jt % rate == rate - 1:
                    heavy.pop(0)()
            # free the o_ps slots fast: the PSUM->SBUF copies go on the DVE
            # queue ahead of the last jt's staging casts, and the
            # reciprocal/broadcast/mul tail is deferred into the next chunk.
            ocs = []
            for h in range(2):
                oc = pool_oc.tile([D + 1, IC], f32, tag="oc", name=f"oc{h}")
                nc.vector.tensor_copy(out=oc[:], in_=o_ps[h][:])
                ocs.append(oc)
            if light:
                light.pop(0)()
            if heavy and (njt - 1) % rate == rate - 1:
                heavy.pop(0)()
            while norm_prev:
                norm_prev.pop(0)()

            # normalize tail as small closures, popped one per jt in the next
            # chunk. reciprocal_approx_fast and partition_broadcast need
            # partition-0 sources, so both heads' Z rows are staged into one
            # [1, 2*IC] tile first.
            st8 = {"rb": [None, None]}

            def p_zz(h):
                if h == 0:
                    st8["zz"] = pool_rz.tile([1, 2, IC], f32, tag="zz", name="zz")
                nc.vector.tensor_copy(out=st8["zz"][0:1, h, :],
                                      in_=ocs[h][D:D + 1, :])

            def p_recip():
                rz = pool_rz.tile([1, 2, IC], f32, tag="rz", name="rz")
                nc.vector.reciprocal_approx_fast(out=rz[:], in_=st8["zz"][:])
                st8["rz"] = rz

            def p_bcast(h):
                rb = pool_rb.tile([D, IC], f32, tag=f"rb{h}", name=f"rb{h}")
                nc.gpsimd.partition_broadcast(rb[:], st8["rz"][0:1, h, :])
                st8["rb"][h] = rb

            def p_mul(h):
                at = ATc[(b, icx)][D * h:D * (h + 1), :]
                nc.vector.tensor_mul(out=at, in0=ocs[h][0:D, :],
                                     in1=st8["rb"][h][:])

            return [lambda: p_zz(0), lambda: p_zz(1), p_recip,
                    lambda: p_bcast(0), lambda: p_bcast(1),
                    lambda: p_mul(0), lambda: p_mul(1)]

        # ---- program ----
        # head: first (tiny) kv window + first q chunk + second window's
        # first chunk, so attention jt 0..4 is runnable almost immediately.
        b0_kv = kv_items(0)
        for it in b0_kv[:2]:
            it()
        proj_q_chunk(0, 0)
        for it in b0_kv[2:4]:
            it()
        b1_kv = kv_items(1)
        heavy = (b0_kv[4:] + [q_item(0, 1)]
                 + b1_kv[:6] + [q_item(0, 2)] + b1_kv[6:] + [q_item(0, 3)]
                 + [q_item(1, icx) for icx in range(NQC)])

        ready, delay, norm2 = [], [], None
        for b in range(B):
            for icx in range(NQC):
                rate = 1 if (b, icx) == (0, 0) else (2 if b == 0 else 3)
                norm2 = attn_chunk(b, icx, ready, delay, heavy, norm2, rate)
                ready.extend(delay)
                delay = outproj_units(b, icx)
        while heavy:
            heavy.pop(0)()
        while ready:                    # already normalized: overlaps norm2
            ready.pop(0)()
        for p in norm2:
            p()
        for u in delay:
            u()

    nc.compile()
    _cache[key] = nc
    return nc


def _prepare(x, attn_mask, Wq, bq, Wk, bk, Wv, bv, Wo):
    mask = np.asarray(attn_mask).astype(bool)
    xf = np.asarray(x, dtype=np.float32).reshape(B, T, E)

    nkt = []
    cols = []       # packed kv token features, (KP, E) f32
    mbcols = []     # per packed slot: 0 keep / -1e30 pad
    for b in range(B):
        idx = np.nonzero(mask[b])[0]
        nk = len(idx)
        ntiles = max(1, (nk + 127) // 128)
        npad = ntiles * 128
        feats = np.zeros((npad, E), dtype=np.float32)
        feats[:nk] = xf[b, idx, :]
        bias = np.full(npad, -1e30, dtype=np.float32)
        bias[:nk] = 0.0
        nkt.append(ntiles)
        cols.append(feats)
        mbcols.append(bias)

    xk = np.ascontiguousarray(np.concatenate(cols, 0).T).astype(_BF16)
    mb_flat = np.concatenate(mbcols)
    ntt = nkt[0] + nkt[1]
    mb = np.ascontiguousarray(mb_flat.reshape(ntt, 128).T)

    xT = np.ascontiguousarray(xf.reshape(BT, E).T).astype(_BF16)
    ident = np.eye(128, dtype=_BF16)

    def wtile(W, sl):
        # [E, PCN] -> [128, EC*PCN] so one DMA loads the whole tile
        wt = np.asarray(W[:, sl], dtype=np.float32).reshape(EC, 128, PCN)
        return np.ascontiguousarray(
            wt.transpose(1, 0, 2).reshape(128, EC * PCN)).astype(_BF16)

    in_maps = []
    for c in range(N_CORES):
        sl = slice(c * PCN, (c + 1) * PCN)
        in_maps.append({
            "xT": xT, "xk": xk, "mb": mb, "ident": ident,
            "wq": wtile(Wq, sl),
            "wk": wtile(Wk, sl),
            "wv": wtile(Wv, sl),
            "wo": np.ascontiguousarray(Wo[sl, :]).astype(_BF16),
            "bq": np.ascontiguousarray(bq[sl]).reshape(PCN, 1).astype(np.float32),
            "bk": np.ascontiguousarray(bk[sl]).reshape(PCN, 1).astype(np.float32),
        })
    return nkt, in_maps


def _run(inputs, trace=False, tmpdir=None):
    from concourse.bass_utils import run_bass_kernel_spmd

    nkt, in_maps = _prepare(
        inputs["x"], inputs["attn_mask"], inputs["Wq"], inputs["bq"],
        inputs["Wk"], inputs["bk"], inputs["Wv"], inputs["bv"], inputs["Wo"])
    nc = _build(nkt)
    res = run_bass_kernel_spmd(nc, in_maps, list(range(N_CORES)),
                               trace=trace, tmpdir=tmpdir)
    yT = np.zeros((E, BT), dtype=np.float64)
    for c in range(N_CORES):
        yT += np.asarray(res.results[c]["yT"], dtype=np.float64)
    # the V bias commutes through attention (sum_k a_k = 1) and the
    # out-projection is linear, so bv lands as a constant bv @ Wo per token.
    yfix = (np.asarray(inputs["bv"], dtype=np.float64)
            @ np.asarray(inputs["Wo"], dtype=np.float64)
            + np.asarray(inputs["bo"], dtype=np.float64))
    y = (yT.T + yfix[None, :]).astype(np.float32)
    return y.reshape(B, T, E), res


def kernel(**inputs):
    y, _ = _run(inputs)
    return y


# revision 25
# speedup vs baseline: 1.0211x; 1.0211x over previous
"""ChaosAttention on 8 Trainium2 NeuronCores.

Sharding: tensor-parallel over heads. Each of the 8 cores owns H/8 = 2 heads
(128 of the 1024 q/k/v projection columns, 128 of the Wo rows). Every core
reads the full x (as x^T, bf16); the out-projection is row-parallel, so each
core returns a partial y^T (bf16) and the host sums the 8 partials and adds
bo plus the folded V-bias term (bv @ Wo).

The physics adapter (physics_state @ Wp1 -> gelu -> @ Wp2) produces a bias
that is constant along the softmax axis, so softmax(x + c) == softmax(x) and
the adapter has zero effect on the output; it is skipped entirely.

Key compaction: masked keys get -inf scores in the reference, so the host
packs only the kept keys per batch (padded to a multiple of 128; pad slots
get a -1e30 exp bias). With a ~50% random mask this halves QK^T / exp / AV.

Device kernel layout (per core):
  - scores are computed transposed (S^T: keys on partitions, queries free) so
    the pad-mask is a per-partition bias fused into the ACT-engine exp, and
    the AV matmul needs no transpose of the probs.
  - the two heads are row-packed into the PE array for QK^T via tile_position.
  - V is produced with wv stationary (big N<=512 matmuls instead of tiny
    N=128 ones) as v^T, then flipped into the [keys, dims] layout AV needs
    with cheap PE transposes (identity matmul). An appended ones-column makes
    the AV matmul also emit the softmax denominator Z in the same PSUM
    accumulation.
  - 1/Z via one merged reciprocal_approx_fast per chunk (~5x faster than DVE
    reciprocal, ~18 good bits; needs a partition-0 source, so both heads' Z
    rows are first staged into a [1, 2*IC] tile). gpsimd broadcasts it across
    the head dims; the V-bias is folded into the host-side output fixup
    (bv @ Wo), so no per-chunk bias add is needed.
  - softmax max-subtraction is skipped (scores are O(1), fp32 exp is safe).
  - y^T partials are staged to bf16 (halves the HBM write traffic); each
    write is split into two partition-halves so the final chunk's writes
    spread over more DMA queues.
  - The PE clock drops to 1.2 GHz after any idle gap and only returns to
    2.4 GHz after 3us of continuous execution, so the whole program is
    emission-ordered to keep the PE's in-order queue non-empty: batch 0 gets
    a tiny 128-key first window so attention starts within ~5us, and all
    remaining projection work is drip-fed into the attention inner loop in
    DMA-arrival order (a per-chunk pre-pop guarantees a chunk's Q projection
    is always emitted before its first QK matmul).
"""

import numpy as np
import ml_dtypes

_BF16 = ml_dtypes.bfloat16

B, T, E, H, D = 2, 2048, 1024, 16, 64
BT = B * T                 # 4096 tokens
N_CORES = 8
PCN = E // N_CORES         # 128 per-core projection dims (2 heads x 64)
EC = E // 128              # 8 contraction chunks for the projections
IC = 512                   # query-chunk size
NQC = T // IC              # 4 query chunks per batch
SCALE = 1.0 / float(np.sqrt(D))

_cache = {}


def _kwins(nkt, b):
    """Key windows for batch b as (start, size) over its packed columns.
    Batch 0 gets a tiny 128-wide first window so attention can start early."""
    n = nkt[b] * 128
    wins = []
    w0 = 0
    if b == 0:
        wins.append((0, 128))
        w0 = 128
    while w0 < n:
        sz = min(512, n - w0)
        wins.append((w0, sz))
        w0 += sz
    return wins


def _build(nkt):
    """Build + schedule the per-core Bass program. nkt = packed key-tile
    counts per batch (ceil(kept/128))."""
    key = tuple(nkt)
    if key in _cache:
        return _cache[key]

    from contextlib import ExitStack
    import concourse.tile as tile
    from concourse import bacc, mybir

    f32 = mybir.dt.float32
    bf16 = mybir.dt.bfloat16
    Exp = mybir.ActivationFunctionType.Exp

    base = [0, nkt[0]]              # packed key-tile offset per batch
    ntt = nkt[0] + nkt[1]           # total packed key tiles
    KP = ntt * 128                  # total packed+padded kv tokens

    kwins = {b: _kwins(nkt, b) for b in range(B)}
    # jt -> (window index, 128-key subtile within window)
    tmap = {b: [(wi, q) for wi, (w0, wsz) in enumerate(kwins[b])
                for q in range(wsz // 128)] for b in range(B)}

    nc = bacc.Bacc("TRN2", target_bir_lowering=False, debug=False,
                   num_devices=N_CORES)

    xT_d = nc.dram_tensor("xT", [E, BT], bf16, kind="ExternalInput").ap()
    xk_d = nc.dram_tensor("xk", [E, KP], bf16, kind="ExternalInput").ap()
    wq_d = nc.dram_tensor("wq", [128, EC * PCN], bf16, kind="ExternalInput").ap()
    wk_d = nc.dram_tensor("wk", [128, EC * PCN], bf16, kind="ExternalInput").ap()
    wv_d = nc.dram_tensor("wv", [128, EC * PCN], bf16, kind="ExternalInput").ap()
    wo_d = nc.dram_tensor("wo", [PCN, E], bf16, kind="ExternalInput").ap()
    bq_d = nc.dram_tensor("bq", [PCN, 1], f32, kind="ExternalInput").ap()
    bk_d = nc.dram_tensor("bk", [PCN, 1], f32, kind="ExternalInput").ap()
    mb_d = nc.dram_tensor("mb", [128, ntt], f32, kind="ExternalInput").ap()
    id_d = nc.dram_tensor("ident", [128, 128], bf16, kind="ExternalInput").ap()
    yT_d = nc.dram_tensor("yT", [E, BT], bf16, kind="ExternalOutput").ap()

    with tile.TileContext(nc) as tc, ExitStack() as ctx:
        consts = ctx.enter_context(tc.tile_pool(name="consts", bufs=1))
        pp_mm = ctx.enter_context(tc.tile_pool(name="ppmm", bufs=2, space="PSUM"))
        pp_st = ctx.enter_context(tc.tile_pool(name="ppst", bufs=2, space="PSUM"))
        pp_o = ctx.enter_context(tc.tile_pool(name="ppo", bufs=2, space="PSUM"))
        pool_pt = ctx.enter_context(tc.tile_pool(name="ptp", bufs=4))
        pool_vt = ctx.enter_context(tc.tile_pool(name="vtp", bufs=2))
        pool_oc = ctx.enter_context(tc.tile_pool(name="ocp", bufs=2))
        pool_rz = ctx.enter_context(tc.tile_pool(name="rzp", bufs=2))
        pool_rb = ctx.enter_context(tc.tile_pool(name="rbp", bufs=2))
        pool_y = ctx.enter_context(tc.tile_pool(name="yp", bufs=4))

        # ---- persistent SBUF residents ----
        xq = [consts.tile([128, EC, IC], bf16, tag=f"xq{mw}", name=f"xq{mw}")
              for mw in range(BT // IC)]
        xkw = {}
        for b in range(B):
            for wi, (w0, wsz) in enumerate(kwins[b]):
                xkw[(b, wi)] = consts.tile([128, EC, wsz], bf16,
                                           tag=f"xk{b}_{wi}", name=f"xk{b}_{wi}")

        wq_sb = consts.tile([128, EC, PCN], bf16, tag="wq")
        wk_sb = consts.tile([128, EC, PCN], bf16, tag="wk")
        wv_sb = consts.tile([128, EC, PCN], bf16, tag="wv")
        wo_sb = consts.tile([128, E], bf16, tag="wo")
        mb_sb = consts.tile([128, ntt], f32, tag="mb")
        bq_sb = consts.tile([128, 1], f32, tag="bq")
        bk_sb = consts.tile([128, 1], f32, tag="bk")
        id_sb = consts.tile([128, 128], bf16, tag="ident")

        # DMA piece sizes balance two costs: each dma_start occupies its
        # issuing engine ~700ns (so startup-critical issues are spread across
        # all four idle engines), and one dma_start = one queue at ~20GB/s
        # (so startup-critical data is split across many queues).
        xkT_v = xk_d.rearrange("(c p) n -> p c n", p=128)
        xTT_v = xT_d.rearrange("(c p) n -> p c n", p=128)
        _rr = {"i": 0}
        _engs = None

        def issue(dst, src):
            if _engs is None:
                nc.sync.dma_start(dst, src)
            else:
                eng = _engs[_rr["i"] % len(_engs)]
                _rr["i"] += 1
                eng.dma_start(dst, src)

        def dma_xk(b, wi, pieces):
            w0, wsz = kwins[b][wi]
            c0 = base[b] * 128 + w0
            step = EC // pieces
            for p in range(pieces):
                issue(xkw[(b, wi)][:, p * step:(p + 1) * step, :],
                      xkT_v[:, p * step:(p + 1) * step, c0:c0 + wsz])

        def dma_xq(mw, pieces):
            step = EC // pieces
            for p in range(pieces):
                issue(xq[mw][:, p * step:(p + 1) * step, :],
                      xTT_v[:, p * step:(p + 1) * step, mw * IC:(mw + 1) * IC])

        def dma_w(w_sb, w_d, pieces=2):
            w_v = w_d.rearrange("p (c n) -> p c n", n=PCN)
            step = EC // pieces
            for p in range(pieces):
                issue(w_sb[:, p * step:(p + 1) * step, :],
                      w_v[:, p * step:(p + 1) * step, :])

        # DMA emission order matches the order the PE will need the data.
        # Startup-critical stream: issue round-robin over all engines.
        _engs = [nc.sync, nc.scalar, nc.gpsimd]
        dma_w(wk_sb, wk_d)
        dma_xk(0, 0, 4)
        issue(id_sb[:], id_d[:])        # v00's PE transposes block on this
        issue(bk_sb[:], bk_d[:])
        dma_w(wq_sb, wq_d)
        dma_w(wv_sb, wv_d)
        issue(mb_sb[:], mb_d[:])
        dma_xq(0, 8)
        issue(bq_sb[:], bq_d[:])
        _engs = [nc.sync, nc.gpsimd]    # scalar starts exps soon after
        dma_xk(0, 1, 8)
        for p in range(2):
            issue(wo_sb[:, p * 512:(p + 1) * 512],
                  wo_d[:, p * 512:(p + 1) * 512])
        dma_xq(1, 8)
        _engs = None                    # prefetch stream: sync only
        for wi in range(2, len(kwins[0])):
            dma_xk(0, wi, 2)
        for wi in range(len(kwins[1])):
            dma_xk(1, wi, 4)
        dma_xq(2, 4)
        dma_xq(3, 4)
        for mw in range(4, BT // IC):
            dma_xq(mw, 2)

        # per-chunk tiles: fine-grained deps let attention start early
        qTc = {(b, icx): consts.tile([128, IC], bf16, tag=f"qT{b}_{icx}",
                                     name=f"qT{b}_{icx}")
               for b in range(B) for icx in range(NQC)}
        ATc = {(b, icx): consts.tile([128, IC], bf16, tag=f"AT{b}_{icx}",
                                     name=f"AT{b}_{icx}")
               for b in range(B) for icx in range(NQC)}
        kTc = {}
        Vpg = {}
        for b in range(B):
            for wi, (w0, wsz) in enumerate(kwins[b]):
                kTc[(b, wi)] = consts.tile([128, wsz], bf16, tag=f"kT{b}_{wi}",
                                           name=f"kT{b}_{wi}")
                Vpg[(b, wi)] = consts.tile([128, wsz // 128, 2, D + 1], bf16,
                                           tag=f"Vp{b}_{wi}", name=f"Vp{b}_{wi}")
                nc.gpsimd.memset(Vpg[(b, wi)][:, :, :, D:D + 1], 1.0)

        # ---- phase emitters ----
        q_emitted = set()

        def proj_q_chunk(b, icx):
            m0 = b * T + icx * IC
            ps = pp_mm.tile([128, 512], f32, tag="mm", name="psq")
            for ec in range(EC):
                nc.tensor.matmul(ps[:], lhsT=wq_sb[:, ec, :],
                                 rhs=xq[m0 // IC][:, ec, :],
                                 start=(ec == 0), stop=(ec == EC - 1))
            nc.vector.tensor_scalar_add(out=qTc[(b, icx)][:], in0=ps[:],
                                        scalar1=bq_sb[:])
            q_emitted.add((b, icx))

        def proj_k_chunk(b, wi, wsz):
            ps = pp_mm.tile([128, 512], f32, tag="mm", name="psk")
            for ec in range(EC):
                nc.tensor.matmul(ps[:, 0:wsz], lhsT=wk_sb[:, ec, :],
                                 rhs=xkw[(b, wi)][:, ec, :],
                                 start=(ec == 0), stop=(ec == EC - 1))
            nc.vector.tensor_scalar_add(out=kTc[(b, wi)][:], in0=ps[:, 0:wsz],
                                        scalar1=bk_sb[:])

        def proj_v_group(b, wi, wsz):
            # v^T = wv^T x (wv stationary, N=wsz), then PE-transpose back to
            # the [keys, dims] layout AV needs.
            nt = wsz // 128
            ps = pp_mm.tile([128, 512], f32, tag="mm", name="psv")
            for ec in range(EC):
                nc.tensor.matmul(ps[:, 0:wsz], lhsT=wv_sb[:, ec, :],
                                 rhs=xkw[(b, wi)][:, ec, :],
                                 start=(ec == 0), stop=(ec == EC - 1))
            vt = pool_vt.tile([128, 512], bf16, tag="vt", name="vt")
            nc.vector.tensor_copy(out=vt[:, 0:wsz], in_=ps[:, 0:wsz])
            tp = pp_mm.tile([128, 512], bf16, tag="mm", name="tpv")
            for q in range(nt):
                nc.tensor.transpose(tp[:, q * 128:(q + 1) * 128],
                                    vt[:, q * 128:(q + 1) * 128], id_sb[:])
            nc.vector.tensor_copy(
                out=Vpg[(b, wi)][:, 0:nt, :, 0:D],
                in_=tp[:, 0:wsz].rearrange("p (q h d) -> p q h d", q=nt, h=2))

        def kv_items(b, skip=0):
            items = []
            for wi, (w0, wsz) in enumerate(kwins[b]):
                items.append(lambda b=b, wi=wi, wsz=wsz: proj_k_chunk(b, wi, wsz))
                items.append(lambda b=b, wi=wi, wsz=wsz: proj_v_group(b, wi, wsz))
            return items[skip:]

        def q_item(b, icx):
            return lambda: proj_q_chunk(b, icx)

        def outproj_unit(b, icx, et, eng):
            yp = pp_mm.tile([128, 512], f32, tag="mm", name="psy")
            nc.tensor.matmul(yp[:], lhsT=wo_sb[:, et * 128:(et + 1) * 128],
                             rhs=ATc[(b, icx)][:], start=True, stop=True)
            ysb = pool_y.tile([128, 512], bf16, tag="y")
            if eng == "act":
                # last chunk only: exps are done, so ACT can help drain
                nc.scalar.activation(out=ysb[:], in_=yp[:],
                                     func=mybir.ActivationFunctionType.Copy)
            else:
                nc.vector.tensor_copy(out=ysb[:], in_=yp[:])
            i0 = b * T + icx * IC
            # issue batch-0 writes from gpsimd (sync is busy with the input
            # stream early on) and batch-1 writes from sync. The last chunk
            # is the kernel tail: alternate sync/gpsimd so neither engine's
            # ~700ns/issue serializes the final drain.
            if b == B - 1 and icx == NQC - 1:
                q = nc.sync if et % 2 else nc.gpsimd
            else:
                q = nc.sync if b else nc.gpsimd
            q.dma_start(yT_d[et * 128:(et + 1) * 128, i0:i0 + 512], ysb[:])

        def outproj_units(b, icx):
            # the last two chunks' units run after the final exp, so ACT can
            # take half their staging casts and drain in parallel with DVE;
            # the first two chunks' units run while attention is PE-paced
            # (ACT has slack there), so ACT takes part of those too.
            tail = b == B - 1 and icx >= NQC - 2
            early = b == 0 and icx < 2
            return [lambda et=et: outproj_unit(
                        b, icx, et,
                        "act" if (tail and et % 2 == icx % 2)
                        or (early and et % 2) else "dve")
                    for et in range(EC)]

        def attn_chunk(b, icx, light, light2, heavy, norm_prev, rate):
            # a chunk's Q projection must be on the PE queue before its QK
            while (b, icx) not in q_emitted and heavy:
                heavy.pop(0)()
            o_ps = [pp_o.tile([D + 1, IC], f32, tag="o", name=f"o{h}")
                    for h in range(2)]
            njt = nkt[b]
            for jt in range(njt):
                tg = base[b] + jt
                wi, q = tmap[b][jt]
                st = pp_st.tile([128, 2 * IC], f32, tag="st")
                nc.tensor.matmul(st[:, 0:IC],
                                 lhsT=kTc[(b, wi)][0:64, q * 128:(q + 1) * 128],
                                 rhs=qTc[(b, icx)][0:64, :],
                                 tile_position=(0, 0), start=True, stop=True)
                nc.tensor.matmul(st[:, IC:2 * IC],
                                 lhsT=kTc[(b, wi)][64:128, q * 128:(q + 1) * 128],
                                 rhs=qTc[(b, icx)][64:128, :],
                                 tile_position=(64, 0), start=True, stop=True)
                pt = pool_pt.tile([128, 2 * IC], bf16, tag="pt")
                nc.scalar.activation(out=pt[:], in_=st[:], func=Exp,
                                     bias=mb_sb[:, tg:tg + 1], scale=SCALE)
                for h in range(2):
                    nc.tensor.matmul(o_ps[h][:], lhsT=Vpg[(b, wi)][:, q, h, :],
                                     rhs=pt[:, h * IC:(h + 1) * IC],
                                     start=(jt == 0), stop=(jt == njt - 1))
                if jt == njt - 1:
                    break               # last jt's pops happen after the ocs
                if light:
                    light.pop(0)()
                if norm_prev and jt >= 2:
                    norm_prev.pop(0)()
                if norm_prev and jt >= 3:
                    norm_prev.pop(0)()
                # only in the last chunk (no more exps to starve): start the
                # previous chunk's units as soon as its normalize is done
                if (light2 and jt >= 6 and not norm_prev
                        and (b, icx) == (B - 1, NQC - 1)):
                    light2.pop(0)()
                if heavy and jt % rate == rate - 1:
                    heavy.pop(0)()
            # free the o_ps slots fast: the PSUM->SBUF copies go on the DVE
            # queue ahead of the last jt's staging casts, and the
            # reciprocal/broadcast/mul tail is deferred into the next chunk.
            ocs = []
            for h in range(2):
                oc = pool_oc.tile([D + 1, IC], f32, tag="oc", name=f"oc{h}")
                nc.vector.tensor_copy(out=oc[:], in_=o_ps[h][:])
                ocs.append(oc)
            if light:
                light.pop(0)()
            if heavy and (njt - 1) % rate == rate - 1:
                heavy.pop(0)()
            while norm_prev:
                norm_prev.pop(0)()

            # normalize tail as small closures, popped one per jt in the next
            # chunk. reciprocal_approx_fast and partition_broadcast need
            # partition-0 sources, so both heads' Z rows are staged into one
            # [1, 2*IC] tile first.
            st8 = {"rb": [None, None]}

            def p_zz(h):
                if h == 0:
                    st8["zz"] = pool_rz.tile([1, 2, IC], f32, tag="zz", name="zz")
                nc.vector.tensor_copy(out=st8["zz"][0:1, h, :],
                                      in_=ocs[h][D:D + 1, :])

            def p_recip():
                rz = pool_rz.tile([1, 2, IC], f32, tag="rz", name="rz")
                nc.vector.reciprocal_approx_fast(out=rz[:], in_=st8["zz"][:])
                st8["rz"] = rz

            def p_bcast(h):
                rb = pool_rb.tile([D, IC], f32, tag=f"rb{h}", name=f"rb{h}")
                nc.gpsimd.partition_broadcast(rb[:], st8["rz"][0:1, h, :])
                st8["rb"][h] = rb

            def p_mul(h):
                at = ATc[(b, icx)][D * h:D * (h + 1), :]
                nc.vector.tensor_mul(out=at, in0=ocs[h][0:D, :],
                                     in1=st8["rb"][h][:])

            return [lambda: p_zz(0), lambda: p_zz(1), p_recip,
                    lambda: p_bcast(0), lambda: p_bcast(1),
                    lambda: p_mul(0), lambda: p_mul(1)]

        # ---- program ----
        # head: first (tiny) kv window + first q chunk + second window's
        # first chunk, so attention jt 0..4 is runnable almost immediately.
        b0_kv = kv_items(0)
        for it in b0_kv[:2]:
            it()
        proj_q_chunk(0, 0)
        for it in b0_kv[2:4]:
            it()
        b1_kv = kv_items(1)
        heavy = (b0_kv[4:] + [q_item(0, 1)]
                 + b1_kv[:6] + [q_item(0, 2)] + b1_kv[6:] + [q_item(0, 3)]
                 + [q_item(1, icx) for icx in range(NQC)])

        ready, delay, norm2 = [], [], None
        for b in range(B):
            for icx in range(NQC):
                rate = 1 if (b, icx) == (0, 0) else (2 if b == 0 else 3)
                norm2 = attn_chunk(b, icx, ready, delay, heavy, norm2, rate)
                ready.extend(delay)
                delay = outproj_units(b, icx)
        while heavy:
            heavy.pop(0)()
        while ready:                    # already normalized: overlaps norm2
            ready.pop(0)()
        for p in norm2:
            p()
        for u in delay:
            u()

    nc.compile()
    _cache[key] = nc
    return nc


def _prepare(x, attn_mask, Wq, bq, Wk, bk, Wv, bv, Wo):
    mask = np.asarray(attn_mask).astype(bool)
    xf = np.asarray(x, dtype=np.float32).reshape(B, T, E)

    nkt = []
    cols = []       # packed kv token features, (KP, E) f32
    mbcols = []     # per packed slot: 0 keep / -1e30 pad
    for b in range(B):
        idx = np.nonzero(mask[b])[0]
        nk = len(idx)
        ntiles = max(1, (nk + 127) // 128)
        npad = ntiles * 128
        feats = np.zeros((npad, E), dtype=np.float32)
        feats[:nk] = xf[b, idx, :]
        bias = np.full(npad, -1e30, dtype=np.float32)
        bias[:nk] = 0.0
        nkt.append(ntiles)
        cols.append(feats)
        mbcols.append(bias)

    xk = np.ascontiguousarray(np.concatenate(cols, 0).T).astype(_BF16)
    mb_flat = np.concatenate(mbcols)
    ntt = nkt[0] + nkt[1]
    mb = np.ascontiguousarray(mb_flat.reshape(ntt, 128).T)

    xT = np.ascontiguousarray(xf.reshape(BT, E).T).astype(_BF16)
    ident = np.eye(128, dtype=_BF16)

    def wtile(W, sl):
        # [E, PCN] -> [128, EC*PCN] so one DMA loads the whole tile
        wt = np.asarray(W[:, sl], dtype=np.float32).reshape(EC, 128, PCN)
        return np.ascontiguousarray(
            wt.transpose(1, 0, 2).reshape(128, EC * PCN)).astype(_BF16)

    in_maps = []
    for c in range(N_CORES):
        sl = slice(c * PCN, (c + 1) * PCN)
        in_maps.append({
            "xT": xT, "xk": xk, "mb": mb, "ident": ident,
            "wq": wtile(Wq, sl),
            "wk": wtile(Wk, sl),
            "wv": wtile(Wv, sl),
            "wo": np.ascontiguousarray(Wo[sl, :]).astype(_BF16),
            "bq": np.ascontiguousarray(bq[sl]).reshape(PCN, 1).astype(np.float32),
            "bk": np.ascontiguousarray(bk[sl]).reshape(PCN, 1).astype(np.float32),
        })
    return nkt, in_maps


def _run(inputs, trace=False, tmpdir=None):
    from concourse.bass_utils import run_bass_kernel_spmd

    nkt, in_maps = _prepare(
        inputs["x"], inputs["attn_mask"], inputs["Wq"], inputs["bq"],
        inputs["Wk"], inputs["bk"], inputs["Wv"], inputs["bv"], inputs["Wo"])
    nc = _build(nkt)
    res = run_bass_kernel_spmd(nc, in_maps, list(range(N_CORES)),
                               trace=trace, tmpdir=tmpdir)
    yT = np.zeros((E, BT), dtype=np.float64)
    for c in range(N_CORES):
        yT += np.asarray(res.results[c]["yT"], dtype=np.float64)
    # the V bias commutes through attention (sum_k a_k = 1) and the
    # out-projection is linear, so bv lands as a constant bv @ Wo per token.
    yfix = (np.asarray(inputs["bv"], dtype=np.float64)
            @ np.asarray(inputs["Wo"], dtype=np.float64)
            + np.asarray(inputs["bo"], dtype=np.float64))
    y = (yT.T + yfix[None, :]).astype(np.float32)
    return y.reshape(B, T, E), res


def kernel(**inputs):
    y, _ = _run(inputs)
    return y
